# revision 1
# baseline (speedup 1.0000x reference)
"""Causal self-attention Bass/Tile kernel for Trainium2, SPMD over 8 NeuronCores.

Problem: B=4, T=2048, C=768, NH=12 heads, D=64. y = softmax(mask(qk^T/sqrt(D))) v,
with qkv = x@W_attn + b_attn and out = y@W_proj + b_proj.

Sharding: core c handles batch b = c//2 and heads [hs, hs+6) where hs = (c%2)*6
(data parallel over batch x tensor parallel over head-halves). Each core computes
a partial output part_c = y_c @ W_proj[rows of its heads]; the host sums the two
partials of each batch pair and adds b_proj (linear ops, exact in fp32).

On-chip dataflow (all matmuls in float32r = TF32-like, ~1e-4 rel err):
  qT[d,t], kT[d,t] per head-pair (2 heads packed on 128 partitions),
  v_aug[t, 6*65] (v columns + a ones column per head -> softmax denominator
  accumulates for free as row 64 of the yT_aug matmul output),
  scoresT[s,t] = kT^T qT via PE (row-tiled: even head on array rows 0:63,
  odd head on 64:127, concurrent), exp on ACT (scale=1/sqrt(D)), causal mask
  via gpsimd affine_select on diagonal blocks, yT_aug[65,t] += v_aug^T P
  accumulated over s-blocks in PSUM, division by the denominator via DVE
  reciprocal + gpsimd partition-broadcast + DVE multiply, projection matmul.

The `loops` build parameter wraps each phase in a hardware For_i loop (used
only by the timing harness; the graded path uses loops=(1,1,1) => no loops).
"""

import contextlib
import os
import sys

for _p in ("/opt/trn_rl_repo", "/root/.axon_site/_ro/trn_rl_repo"):
    if os.path.isdir(_p) and _p not in sys.path:
        sys.path.insert(0, _p)
        break

import numpy as np

import concourse.bass as bass  # noqa: F401
import concourse.mybir as mybir
import concourse.tile as tile
from concourse import bacc
from concourse.bass_utils import run_bass_kernel_spmd

FP32 = mybir.dt.float32
FP32R = mybir.dt.float32r

B, T, C = 4, 2048, 768
NH, D = 12, 64
NCORES = 8
NKC = C // 128          # 6 contraction chunks for qkv
NTB = T // 128          # 16 t blocks
TCW = 512
NTC = T // TCW          # 4 t chunks
HPC = 6                 # heads per core
VW = HPC * 65           # v_aug row width (6 heads x (64 + ones col))
WQKV = 3 * HPC * D      # 1152

_BUILT = None


def _phase_a(nc, tc, loop_cm, la, *, qT, kT, vaug, xT, wqkv, bqk_sb, bvb_sb):
    with tc.sbuf_pool(name="inp", bufs=1) as inp, \
         tc.psum_pool(name="psA", bufs=1) as psA:
        xt = inp.tile([128, NKC * T], FP32R)
        wa = inp.tile([128, NKC * WQKV], FP32R)
        with loop_cm(la):
            for kc in range(NKC):
                nc.sync.dma_start(
                    out=xt[:, kc * T:(kc + 1) * T],
                    in_=xT[kc * 128:(kc + 1) * 128, :],
                )
                nc.sync.dma_start(
                    out=wa[:, kc * WQKV:(kc + 1) * WQKV],
                    in_=wqkv[kc * 128:(kc + 1) * 128, :],
                )
            for qk in range(2):  # 0 -> q, 1 -> k
                dst = qT if qk == 0 else kT
                for pp in range(3):
                    for tcw in range(NTC):
                        acc = psA.tile([128, TCW], FP32, tag="qkp", bufs=2)
                        for kc in range(NKC):
                            co = kc * WQKV + qk * 384 + pp * 128
                            nc.tensor.matmul(
                                acc,
                                wa[:, co:co + 128],
                                xt[:, kc * T + tcw * TCW: kc * T + (tcw + 1) * TCW],
                                start=(kc == 0),
                                stop=(kc == NKC - 1),
                            )
                        nc.vector.tensor_scalar_add(
                            dst[:, pp * T + tcw * TCW: pp * T + (tcw + 1) * TCW],
                            acc,
                            bqk_sb[:, 3 * qk + pp: 3 * qk + pp + 1],
                        )
            for tb in range(NTB):
                accv = psA.tile([128, 384], FP32, tag="vp", bufs=2)
                for kc in range(NKC):
                    nc.tensor.matmul(
                        accv,
                        xt[:, kc * T + tb * 128: kc * T + (tb + 1) * 128],
                        wa[:, kc * WQKV + 768: kc * WQKV + WQKV],
                        start=(kc == 0),
                        stop=(kc == NKC - 1),
                    )
                vdst = vaug[:, tb * VW:(tb + 1) * VW]
                nc.vector.tensor_tensor(
                    out=vdst.rearrange("p (h c) -> p h c", c=65)[:, :, 0:64],
                    in0=accv.rearrange("p (h c) -> p h c", c=64),
                    in1=bvb_sb.rearrange("p (h c) -> p h c", c=64),
                    op=mybir.AluOpType.add,
                )


def _phase_b(nc, tc, loop_cm, lb, npairs, mask, do_exp, *, qT, kT, vaug, yT,
             do_div=True, ptbufs=3, ytbufs=2):
    with tc.sbuf_pool(name="work", bufs=1) as work, \
         tc.psum_pool(name="psB", bufs=1) as psB:
        with loop_cm(lb if npairs else 1):
            for pp in range(npairs):
                for tcw in range(NTC):
                    nsb = 4 * tcw + 4  # s blocks 0 .. 4*tcw+3 (causal)
                    yps = []
                    for e in range(2):
                        ytile = psB.tile([65, TCW], FP32, tag="yt", bufs=ytbufs)
                        yps.append(ytile)
                    for g in range(nsb // 2):
                        for e in range(2):
                            pt2 = psB.tile([128, 2 * TCW], FP32, tag="pt", bufs=ptbufs)
                            for j in range(2):
                                sbi = 2 * g + j
                                nc.tensor.matmul(
                                    pt2[:, j * TCW:(j + 1) * TCW],
                                    kT[e * 64:(e + 1) * 64,
                                       pp * T + sbi * 128: pp * T + (sbi + 1) * 128],
                                    qT[e * 64:(e + 1) * 64,
                                       pp * T + tcw * TCW: pp * T + (tcw + 1) * TCW],
                                    start=True,
                                    stop=True,
                                )
                            pt2sb = work.tile([128, 2 * TCW], qT.dtype, tag="ptsb", bufs=3)
                            if do_exp:
                                nc.scalar.activation(
                                    pt2sb, pt2, mybir.ActivationFunctionType.Exp,
                                    scale=0.125,
                                )
                            else:
                                nc.vector.tensor_copy(pt2sb, pt2)
                            for j in range(2):
                                sbi = 2 * g + j
                                psl = pt2sb[:, j * TCW:(j + 1) * TCW]
                                if mask and sbi >= 4 * tcw:  # diagonal-band block
                                    nc.gpsimd.affine_select(
                                        out=psl, in_=psl,
                                        compare_op=mybir.AluOpType.is_ge,
                                        fill=0.0,
                                        base=tcw * TCW - sbi * 128,
                                        channel_multiplier=-1,
                                        pattern=[[1, TCW]],
                                    )
                                h = 2 * pp + e
                                nc.tensor.matmul(
                                    yps[e],
                                    vaug[:, sbi * VW + h * 65: sbi * VW + (h + 1) * 65],
                                    psl,
                                    start=(sbi == 0),
                                    stop=(sbi == nsb - 1),
                                )
                    for e in range(2):
                        if not do_div:
                            with nc.allow_low_precision(reason="timing variant"):
                                nc.vector.tensor_copy(
                                    yT[e * 64:(e + 1) * 64,
                                       pp * T + tcw * TCW: pp * T + (tcw + 1) * TCW],
                                    yps[e][0:64, :])
                            continue
                        rt = work.tile([1, TCW], FP32R, tag="rt", bufs=2)
                        with nc.allow_low_precision(reason="fp32r softmax denom"):
                            nc.vector.reciprocal(rt, yps[e][64:65, :])
                        rbc = work.tile([64, TCW], FP32R, tag="rbc", bufs=2)
                        nc.gpsimd.partition_broadcast(rbc, rt)
                        with nc.allow_low_precision(reason="fp32r attn out"):
                            nc.vector.tensor_tensor(
                                out=yT[e * 64:(e + 1) * 64,
                                       pp * T + tcw * TCW: pp * T + (tcw + 1) * TCW],
                                in0=yps[e][0:64, :],
                                in1=rbc,
                                op=mybir.AluOpType.mult,
                            )


def _phase_c(nc, tc, loop_cm, lc, ntb, *, yT, wp, part):
    with tc.sbuf_pool(name="outp", bufs=1) as outp, \
         tc.psum_pool(name="psC", bufs=1) as psC:
        with loop_cm(lc if ntb else 1):
            for tb in range(ntb):
                osb = outp.tile([128, C], FP32, tag="osb", bufs=3)
                for ncw in range(2):
                    acc = psC.tile([128, 384], FP32, tag="op", bufs=2)
                    for cc in range(3):
                        nc.tensor.matmul(
                            acc,
                            yT[:, cc * T + tb * 128: cc * T + (tb + 1) * 128],
                            wp[:, cc * C + ncw * 384: cc * C + (ncw + 1) * 384],
                            start=(cc == 0),
                            stop=(cc == 2),
                        )
                    nc.vector.tensor_copy(osb[:, ncw * 384:(ncw + 1) * 384], acc)
                nc.sync.dma_start(
                    out=part[tb * 128:(tb + 1) * 128, :], in_=osb
                )


def _build_nc(phases="ABC", mask=True, do_exp=True, loops=(1, 1, 1),
              bdt=None, do_div=True, ptbufs=3, ytbufs=2):
    if bdt is None:
        bdt = FP32R
    nc = bacc.Bacc("TRN2", target_bir_lowering=False, debug=False, num_devices=NCORES)

    xT = nc.dram_tensor("xT", [C, T], FP32R, kind="ExternalInput")
    wqkv = nc.dram_tensor("wqkv", [C, WQKV], FP32R, kind="ExternalInput")
    bqk = nc.dram_tensor("bqk", [128, 6], FP32, kind="ExternalInput")
    bvb = nc.dram_tensor("bvb", [128, 384], FP32, kind="ExternalInput")
    wproj = nc.dram_tensor("wproj", [384, C], FP32R, kind="ExternalInput")
    part = nc.dram_tensor("part", [T, C], FP32, kind="ExternalOutput")

    with tile.TileContext(nc) as tc:
        def loop_cm(n):
            return tc.For_i(0, n, 1) if n > 1 else contextlib.nullcontext()

        with tc.sbuf_pool(name="pers", bufs=1) as pers:
            qT = pers.tile([128, 3 * T], bdt)
            kT = pers.tile([128, 3 * T], bdt)
            vaug = pers.tile([128, NTB * VW], bdt)
            yT = pers.tile([128, 3 * T], FP32R)
            wp = pers.tile([128, 3 * C], FP32R)
            bqk_sb = pers.tile([128, 6], FP32)
            bvb_sb = pers.tile([128, 384], FP32)

            nc.sync.dma_start(
                out=wp.rearrange("p (k c) -> p k c", c=C),
                in_=wproj.rearrange("(k p) c -> p k c", p=128),
            )
            nc.sync.dma_start(out=bqk_sb, in_=bqk[:, :])
            nc.sync.dma_start(out=bvb_sb, in_=bvb[:, :])
            ones_cols = vaug.rearrange("p (g c) -> p g c", c=65)[:, :, 64:65]
            if bdt == FP32R:
                ones_cols = ones_cols.bitcast(FP32)
            nc.vector.memset(ones_cols, 1.0)

            _phase_a(nc, tc, loop_cm, loops[0], qT=qT, kT=kT, vaug=vaug,
                     xT=xT, wqkv=wqkv, bqk_sb=bqk_sb, bvb_sb=bvb_sb)
            _phase_b(nc, tc, loop_cm, loops[1],
                     npairs=3 if "B" in phases else 0, mask=mask, do_exp=do_exp,
                     qT=qT, kT=kT, vaug=vaug, yT=yT,
                     do_div=do_div, ptbufs=ptbufs, ytbufs=ytbufs)
            _phase_c(nc, tc, loop_cm, loops[2],
                     ntb=NTB if ("C" in phases and "B" in phases) else 0,
                     yT=yT, wp=wp, part=part)

            if "C" not in phases or "B" not in phases:
                with tc.sbuf_pool(name="dummy", bufs=1) as dp:
                    z = dp.tile([128, 512], FP32)
                    nc.vector.memset(z, 0.0)
                    nc.sync.dma_start(out=part[0:128, 0:512], in_=z)

    nc.compile()
    return nc


def _get_nc():
    global _BUILT
    if _BUILT is None:
        _BUILT = _build_nc()
    return _BUILT


def kernel(x, W_attn, b_attn, W_proj, b_proj):
    x = np.asarray(x, dtype=np.float32)
    W_attn = np.asarray(W_attn, dtype=np.float32)
    b_attn = np.asarray(b_attn, dtype=np.float32)
    W_proj = np.asarray(W_proj, dtype=np.float32)
    b_proj = np.asarray(b_proj, dtype=np.float32)

    nc = _get_nc()

    in_maps = []
    for c in range(NCORES):
        b = c // 2
        hs = (c % 2) * HPC
        q0, k0, v0 = hs * D, C + hs * D, 2 * C + hs * D
        w = HPC * D  # 384
        xT_b = np.ascontiguousarray(x[b].T)
        wqkv_c = np.ascontiguousarray(
            np.concatenate(
                [W_attn[:, q0:q0 + w], W_attn[:, k0:k0 + w], W_attn[:, v0:v0 + w]],
                axis=1,
            )
        )
        bqk_c = np.stack(
            [b_attn[q0 + pp * 128: q0 + (pp + 1) * 128] for pp in range(3)]
            + [b_attn[k0 + pp * 128: k0 + (pp + 1) * 128] for pp in range(3)],
            axis=1,
        ).astype(np.float32)
        bvb_c = np.ascontiguousarray(
            np.broadcast_to(b_attn[v0:v0 + w][None, :], (128, w))
        ).astype(np.float32)
        wproj_c = np.ascontiguousarray(W_proj[hs * D: hs * D + w, :])
        in_maps.append(
            {
                "xT": xT_b,
                "wqkv": wqkv_c,
                "bqk": bqk_c,
                "bvb": bvb_c,
                "wproj": wproj_c,
            }
        )

    res = run_bass_kernel_spmd(nc, in_maps, core_ids=list(range(NCORES)))
    out = np.empty((B, T, C), dtype=np.float32)
    for b in range(B):
        out[b] = (
            res.results[2 * b]["part"]
            + res.results[2 * b + 1]["part"]
            + b_proj[None, :]
        )
    return out



# revision 3
# speedup vs baseline: 6051.6575x; 6051.6575x over previous
"""Causal self-attention Bass/Tile kernel for Trainium2, SPMD over 8 NeuronCores.

Problem: B=4, T=2048, C=768, NH=12 heads, D=64. y = softmax(mask(qk^T/sqrt(D))) v,
with qkv = x@W_attn + b_attn and out = y@W_proj + b_proj.

Sharding: core c handles batch b = c//2 and heads [hs, hs+6) where hs = (c%2)*6
(data parallel over batch x tensor parallel over head-halves). Each core computes
a partial output part_c = y_c @ W_proj[rows of its heads]; the host sums the two
partials of each batch pair and adds b_proj (linear ops, exact in fp32).

On-chip dataflow (all matmuls in float32r = TF32-like, ~1e-4 rel err):
  qT[d,t], kT[d,t] per head-pair (2 heads packed on 128 partitions),
  v_aug[t, 6*65] (v columns + a ones column per head -> softmax denominator
  accumulates for free as row 64 of the yT_aug matmul output),
  scoresT[s,t] = kT^T qT via PE (row-tiled: even head on array rows 0:63,
  odd head on 64:127, concurrent), exp on ACT (scale=1/sqrt(D)), causal mask
  via gpsimd affine_select on diagonal blocks, yT_aug[65,t] += v_aug^T P
  accumulated over s-blocks in PSUM, division by the denominator via DVE
  reciprocal + gpsimd partition-broadcast + DVE multiply, projection matmul.

The `loops` build parameter wraps each phase in a hardware For_i loop (used
only by the timing harness; the graded path uses loops=(1,1,1) => no loops).
"""

import contextlib
import os
import sys

for _p in ("/opt/trn_rl_repo", "/root/.axon_site/_ro/trn_rl_repo"):
    if os.path.isdir(_p) and _p not in sys.path:
        sys.path.insert(0, _p)
        break

import numpy as np

import concourse.bass as bass  # noqa: F401
import concourse.mybir as mybir
import concourse.tile as tile
from concourse import bacc
from concourse.bass_utils import run_bass_kernel_spmd

FP32 = mybir.dt.float32
FP32R = mybir.dt.float32r

B, T, C = 4, 2048, 768
NH, D = 12, 64
NCORES = 8
NKC = C // 128          # 6 contraction chunks for qkv
NTB = T // 128          # 16 t blocks
TCW = 512
NTC = T // TCW          # 4 t chunks
HPC = 6                 # heads per core
VW = HPC * 65           # v_aug row width (6 heads x (64 + ones col))
WQKV = 3 * HPC * D      # 1152

_BUILT = None


def _phase_a(nc, tc, loop_cm, la, *, qT, kT, vaug, xT, wqkv, bqk_sb, bvb_sb):
    with tc.sbuf_pool(name="inp", bufs=1) as inp, \
         tc.psum_pool(name="psA", bufs=1) as psA:
        xt = inp.tile([128, NKC * T], FP32R)
        wa = inp.tile([128, NKC * WQKV], FP32R)
        with loop_cm(la):
            for kc in range(NKC):
                nc.sync.dma_start(
                    out=xt[:, kc * T:(kc + 1) * T],
                    in_=xT[kc * 128:(kc + 1) * 128, :],
                )
                nc.sync.dma_start(
                    out=wa[:, kc * WQKV:(kc + 1) * WQKV],
                    in_=wqkv[kc * 128:(kc + 1) * 128, :],
                )
            for qk in range(2):  # 0 -> q, 1 -> k
                dst = qT if qk == 0 else kT
                for pp in range(3):
                    for tcw in range(NTC):
                        acc = psA.tile([128, TCW], FP32, tag="qkp", bufs=2)
                        for kc in range(NKC):
                            co = kc * WQKV + qk * 384 + pp * 128
                            nc.tensor.matmul(
                                acc,
                                wa[:, co:co + 128],
                                xt[:, kc * T + tcw * TCW: kc * T + (tcw + 1) * TCW],
                                start=(kc == 0),
                                stop=(kc == NKC - 1),
                            )
                        nc.vector.tensor_scalar_add(
                            dst[:, pp * T + tcw * TCW: pp * T + (tcw + 1) * TCW],
                            acc,
                            bqk_sb[:, 3 * qk + pp: 3 * qk + pp + 1],
                        )
            for tb in range(NTB):
                accv = psA.tile([128, 384], FP32, tag="vp", bufs=2)
                for kc in range(NKC):
                    nc.tensor.matmul(
                        accv,
                        xt[:, kc * T + tb * 128: kc * T + (tb + 1) * 128],
                        wa[:, kc * WQKV + 768: kc * WQKV + WQKV],
                        start=(kc == 0),
                        stop=(kc == NKC - 1),
                    )
                vdst = vaug[:, tb * VW:(tb + 1) * VW]
                nc.vector.tensor_tensor(
                    out=vdst.rearrange("p (h c) -> p h c", c=65)[:, :, 0:64],
                    in0=accv.rearrange("p (h c) -> p h c", c=64),
                    in1=bvb_sb.rearrange("p (h c) -> p h c", c=64),
                    op=mybir.AluOpType.add,
                )


def _phase_b(nc, tc, loop_cm, lb, npairs, mask, do_exp, *, qT, kT, vaug, yT,
             do_div=True, ptbufs=3, ytbufs=2):
    with tc.sbuf_pool(name="work", bufs=1) as work, \
         tc.psum_pool(name="psB", bufs=1) as psB:
        with loop_cm(lb if npairs else 1):
            for pp in range(npairs):
                for tcw in range(NTC):
                    nsb = 4 * tcw + 4  # s blocks 0 .. 4*tcw+3 (causal)
                    yps = []
                    for e in range(2):
                        ytile = psB.tile([65, TCW], FP32, tag="yt", bufs=ytbufs)
                        yps.append(ytile)
                    for g in range(nsb // 2):
                        for e in range(2):
                            pt2 = psB.tile([128, 2 * TCW], FP32, tag="pt", bufs=ptbufs)
                            for j in range(2):
                                sbi = 2 * g + j
                                nc.tensor.matmul(
                                    pt2[:, j * TCW:(j + 1) * TCW],
                                    kT[e * 64:(e + 1) * 64,
                                       pp * T + sbi * 128: pp * T + (sbi + 1) * 128],
                                    qT[e * 64:(e + 1) * 64,
                                       pp * T + tcw * TCW: pp * T + (tcw + 1) * TCW],
                                    start=True,
                                    stop=True,
                                )
                            pt2sb = work.tile([128, 2 * TCW], qT.dtype, tag="ptsb", bufs=3)
                            if do_exp:
                                nc.scalar.activation(
                                    pt2sb, pt2, mybir.ActivationFunctionType.Exp,
                                    scale=0.125,
                                )
                            else:
                                nc.vector.tensor_copy(pt2sb, pt2)
                            for j in range(2):
                                sbi = 2 * g + j
                                psl = pt2sb[:, j * TCW:(j + 1) * TCW]
                                if mask and sbi >= 4 * tcw:  # diagonal-band block
                                    nc.gpsimd.affine_select(
                                        out=psl, in_=psl,
                                        compare_op=mybir.AluOpType.is_ge,
                                        fill=0.0,
                                        base=tcw * TCW - sbi * 128,
                                        channel_multiplier=-1,
                                        pattern=[[1, TCW]],
                                    )
                                h = 2 * pp + e
                                nc.tensor.matmul(
                                    yps[e],
                                    vaug[:, sbi * VW + h * 65: sbi * VW + (h + 1) * 65],
                                    psl,
                                    start=(sbi == 0),
                                    stop=(sbi == nsb - 1),
                                )
                    for e in range(2):
                        if not do_div:
                            with nc.allow_low_precision(reason="timing variant"):
                                nc.vector.tensor_copy(
                                    yT[e * 64:(e + 1) * 64,
                                       pp * T + tcw * TCW: pp * T + (tcw + 1) * TCW],
                                    yps[e][0:64, :])
                            continue
                        rt = work.tile([1, TCW], FP32R, tag="rt", bufs=2)
                        with nc.allow_low_precision(reason="fp32r softmax denom"):
                            nc.vector.reciprocal(rt, yps[e][64:65, :])
                        rbc = work.tile([64, TCW], FP32R, tag="rbc", bufs=2)
                        nc.gpsimd.partition_broadcast(rbc, rt)
                        with nc.allow_low_precision(reason="fp32r attn out"):
                            nc.vector.tensor_tensor(
                                out=yT[e * 64:(e + 1) * 64,
                                       pp * T + tcw * TCW: pp * T + (tcw + 1) * TCW],
                                in0=yps[e][0:64, :],
                                in1=rbc,
                                op=mybir.AluOpType.mult,
                            )


def _phase_c(nc, tc, loop_cm, lc, ntb, *, yT, wp, part):
    with tc.sbuf_pool(name="outp", bufs=1) as outp, \
         tc.psum_pool(name="psC", bufs=1) as psC:
        with loop_cm(lc if ntb else 1):
            for tb in range(ntb):
                osb = outp.tile([128, C], FP32, tag="osb", bufs=3)
                for ncw in range(2):
                    acc = psC.tile([128, 384], FP32, tag="op", bufs=2)
                    for cc in range(3):
                        nc.tensor.matmul(
                            acc,
                            yT[:, cc * T + tb * 128: cc * T + (tb + 1) * 128],
                            wp[:, cc * C + ncw * 384: cc * C + (ncw + 1) * 384],
                            start=(cc == 0),
                            stop=(cc == 2),
                        )
                    nc.vector.tensor_copy(osb[:, ncw * 384:(ncw + 1) * 384], acc)
                nc.sync.dma_start(
                    out=part[tb * 128:(tb + 1) * 128, :], in_=osb
                )


def _build_nc(phases="ABC", mask=True, do_exp=True, loops=(1, 1, 1),
              bdt=None, do_div=True, ptbufs=3, ytbufs=2):
    if bdt is None:
        bdt = FP32R
    nc = bacc.Bacc("TRN2", target_bir_lowering=False, debug=False, num_devices=NCORES)

    xT = nc.dram_tensor("xT", [C, T], FP32R, kind="ExternalInput")
    wqkv = nc.dram_tensor("wqkv", [C, WQKV], FP32R, kind="ExternalInput")
    bqk = nc.dram_tensor("bqk", [128, 6], FP32, kind="ExternalInput")
    bvb = nc.dram_tensor("bvb", [128, 384], FP32, kind="ExternalInput")
    wproj = nc.dram_tensor("wproj", [384, C], FP32R, kind="ExternalInput")
    part = nc.dram_tensor("part", [T, C], FP32, kind="ExternalOutput")

    with tile.TileContext(nc) as tc:
        def loop_cm(n):
            return tc.For_i(0, n, 1) if n > 1 else contextlib.nullcontext()

        with tc.sbuf_pool(name="pers", bufs=1) as pers:
            qT = pers.tile([128, 3 * T], bdt)
            kT = pers.tile([128, 3 * T], bdt)
            vaug = pers.tile([128, NTB * VW], bdt)
            yT = pers.tile([128, 3 * T], FP32R)
            wp = pers.tile([128, 3 * C], FP32R)
            bqk_sb = pers.tile([128, 6], FP32)
            bvb_sb = pers.tile([128, 384], FP32)

            nc.sync.dma_start(
                out=wp.rearrange("p (k c) -> p k c", c=C),
                in_=wproj.rearrange("(k p) c -> p k c", p=128),
            )
            nc.sync.dma_start(out=bqk_sb, in_=bqk[:, :])
            nc.sync.dma_start(out=bvb_sb, in_=bvb[:, :])
            ones_cols = vaug.rearrange("p (g c) -> p g c", c=65)[:, :, 64:65]
            if bdt == FP32R:
                ones_cols = ones_cols.bitcast(FP32)
            nc.vector.memset(ones_cols, 1.0)

            _phase_a(nc, tc, loop_cm, loops[0], qT=qT, kT=kT, vaug=vaug,
                     xT=xT, wqkv=wqkv, bqk_sb=bqk_sb, bvb_sb=bvb_sb)
            _phase_b(nc, tc, loop_cm, loops[1],
                     npairs=3 if "B" in phases else 0, mask=mask, do_exp=do_exp,
                     qT=qT, kT=kT, vaug=vaug, yT=yT,
                     do_div=do_div, ptbufs=ptbufs, ytbufs=ytbufs)
            _phase_c(nc, tc, loop_cm, loops[2],
                     ntb=NTB if ("C" in phases and "B" in phases) else 0,
                     yT=yT, wp=wp, part=part)

            if "C" not in phases or "B" not in phases:
                with tc.sbuf_pool(name="dummy", bufs=1) as dp:
                    z = dp.tile([128, 512], FP32)
                    nc.vector.memset(z, 0.0)
                    nc.sync.dma_start(out=part[0:128, 0:512], in_=z)

    nc.compile()
    return nc


def _get_nc():
    global _BUILT
    if _BUILT is None:
        _BUILT = _build_nc()
    return _BUILT


def _build_in_maps(x, W_attn, b_attn, W_proj):
    in_maps = []
    for c in range(NCORES):
        b = c // 2
        hs = (c % 2) * HPC
        q0, k0, v0 = hs * D, C + hs * D, 2 * C + hs * D
        w = HPC * D  # 384
        xT_b = np.ascontiguousarray(x[b].T)
        wqkv_c = np.ascontiguousarray(
            np.concatenate(
                [W_attn[:, q0:q0 + w], W_attn[:, k0:k0 + w], W_attn[:, v0:v0 + w]],
                axis=1,
            )
        )
        bqk_c = np.stack(
            [b_attn[q0 + pp * 128: q0 + (pp + 1) * 128] for pp in range(3)]
            + [b_attn[k0 + pp * 128: k0 + (pp + 1) * 128] for pp in range(3)],
            axis=1,
        ).astype(np.float32)
        bvb_c = np.ascontiguousarray(
            np.broadcast_to(b_attn[v0:v0 + w][None, :], (128, w))
        ).astype(np.float32)
        wproj_c = np.ascontiguousarray(W_proj[hs * D: hs * D + w, :])
        in_maps.append(
            {
                "xT": xT_b,
                "wqkv": wqkv_c,
                "bqk": bqk_c,
                "bvb": bvb_c,
                "wproj": wproj_c,
            }
        )
    return in_maps


def kernel(x, W_attn, b_attn, W_proj, b_proj):
    x = np.asarray(x, dtype=np.float32)
    W_attn = np.asarray(W_attn, dtype=np.float32)
    b_attn = np.asarray(b_attn, dtype=np.float32)
    W_proj = np.asarray(W_proj, dtype=np.float32)
    b_proj = np.asarray(b_proj, dtype=np.float32)

    nc = _get_nc()
    in_maps = _build_in_maps(x, W_attn, b_attn, W_proj)

    res = run_bass_kernel_spmd(nc, in_maps, core_ids=list(range(NCORES)))
    out = np.empty((B, T, C), dtype=np.float32)
    for b in range(B):
        out[b] = (
            res.results[2 * b]["part"]
            + res.results[2 * b + 1]["part"]
            + b_proj[None, :]
        )
    return out



# revision 7
# speedup vs baseline: 8156.2100x; 1.3478x over previous
"""Causal self-attention Bass/Tile kernel for Trainium2, SPMD over 8 NeuronCores.

Problem: B=4, T=2048, C=768, NH=12 heads, D=64. y = softmax(mask(qk^T/sqrt(D))) v,
with qkv = x@W_attn + b_attn and out = y@W_proj + b_proj.

Sharding: core c handles batch b = c//2 and heads [hs, hs+6) where hs = (c%2)*6
(data parallel over batch x tensor parallel over head-halves). Each core computes
a partial output part_c = y_c @ W_proj[rows of its heads]; the host sums the two
partials of each batch pair and adds b_proj (linear ops, exact in fp32).

Dataflow: one fused pipeline over t-chunks (tcw of 512). Per chunk: qkv
projection (fp32r), scores kT^T qT row-tiled per head pair (fp32r), exp on ACT
(scale=1/sqrt(D)) to bf16, causal mask via gpsimd affine_select on diagonal
blocks, PV accumulation in PSUM with a ones column per head so the softmax
denominator falls out of the matmul, fast-approx reciprocal + partition
broadcast + multiply for the normalization, then the projection matmul with
bf16 weights DMA'd straight from PSUM to DRAM. Tensors are split per
(pair, t-chunk) so the Tile scheduler can overlap phases.
"""

import os
import sys

for _p in ("/opt/trn_rl_repo", "/root/.axon_site/_ro/trn_rl_repo"):
    if os.path.isdir(_p) and _p not in sys.path:
        sys.path.insert(0, _p)
        break

import numpy as np
import ml_dtypes

import concourse.bass as bass  # noqa: F401
import concourse.mybir as mybir
import concourse.tile as tile
from concourse import bacc
from concourse.bass_utils import run_bass_kernel_spmd

FP32 = mybir.dt.float32
FP32R = mybir.dt.float32r
BF16 = mybir.dt.bfloat16

B, T, C = 4, 2048, 768
NH, D = 12, 64
NCORES = 8
NKC = C // 128          # 6 contraction chunks for qkv
TCW = 512
NTC = T // TCW          # 4 t chunks
HPC = 6                 # heads per core
NPAIR = 3               # head pairs per core
WQKV = 3 * HPC * D      # 1152

_BUILT = None


def _build_nc():
    nc = bacc.Bacc("TRN2", target_bir_lowering=False, debug=False, num_devices=NCORES)

    xT = nc.dram_tensor("xT", [C, T], FP32R, kind="ExternalInput")
    wqkv = nc.dram_tensor("wqkv", [C, WQKV], FP32R, kind="ExternalInput")
    bqk = nc.dram_tensor("bqk", [128, 6], FP32, kind="ExternalInput")
    bvb = nc.dram_tensor("bvb", [128, 384], FP32, kind="ExternalInput")
    wproj = nc.dram_tensor("wproj", [384, C], BF16, kind="ExternalInput")
    part = nc.dram_tensor("part", [T, C], FP32, kind="ExternalOutput")

    with tile.TileContext(nc) as tc:
        with tc.sbuf_pool(name="pers", bufs=1) as pers, \
             tc.sbuf_pool(name="work", bufs=1) as work, \
             tc.psum_pool(name="ps", bufs=1) as ps:
            xt = pers.tile([128, NKC * T], FP32R)
            wa = pers.tile([128, NKC * WQKV], FP32R)
            wp = pers.tile([128, 3 * C], BF16)
            bqk_sb = pers.tile([128, 6], FP32)
            bvb_sb = pers.tile([128, 384], FP32)
            qT = [[pers.tile([128, TCW], FP32R, tag=f"qT{p}_{t}", name=f"qT{p}_{t}")
                   for t in range(NTC)] for p in range(NPAIR)]
            kT = [[pers.tile([128, TCW], FP32R, tag=f"kT{p}_{t}", name=f"kT{p}_{t}")
                   for t in range(NTC)] for p in range(NPAIR)]
            vg = [[pers.tile([128, 8 * 65], BF16, tag=f"vg{p}_{t}", name=f"vg{p}_{t}")
                   for t in range(NTC)] for p in range(NPAIR)]
            yT = [[pers.tile([128, TCW], BF16, tag=f"yT{p}_{t}", name=f"yT{p}_{t}")
                   for t in range(NTC)] for p in range(NPAIR)]

            nc.sync.dma_start(
                out=wp.rearrange("p (k c) -> p k c", c=C),
                in_=wproj.rearrange("(k p) c -> p k c", p=128),
            )
            nc.sync.dma_start(out=bqk_sb, in_=bqk[:, :])
            nc.sync.dma_start(out=bvb_sb, in_=bvb[:, :])
            for kc in range(NKC):
                nc.sync.dma_start(
                    out=wa[:, kc * WQKV:(kc + 1) * WQKV],
                    in_=wqkv[kc * 128:(kc + 1) * 128, :],
                )
            for tcw in range(NTC):
                for kc in range(NKC):
                    nc.sync.dma_start(
                        out=xt[:, kc * T + tcw * TCW: kc * T + (tcw + 1) * TCW],
                        in_=xT[kc * 128:(kc + 1) * 128, tcw * TCW:(tcw + 1) * TCW],
                    )
            for p in range(NPAIR):
                for t in range(NTC):
                    ones_cols = vg[p][t].rearrange("p (g c) -> p g c", c=65)[:, :, 64:65]
                    nc.vector.memset(ones_cols, 1.0)

            for tcw in range(NTC):
                # ---- qkv projection for this t chunk ----
                for pp in range(NPAIR):
                    for qk in range(2):  # 0 -> q, 1 -> k
                        dst = (qT if qk == 0 else kT)[pp][tcw]
                        acc = ps.tile([128, TCW], FP32, tag="a", bufs=2)
                        for kc in range(NKC):
                            co = kc * WQKV + qk * 384 + pp * 128
                            nc.tensor.matmul(
                                acc,
                                wa[:, co:co + 128],
                                xt[:, kc * T + tcw * TCW: kc * T + (tcw + 1) * TCW],
                                start=(kc == 0),
                                stop=(kc == NKC - 1),
                            )
                        nc.vector.tensor_scalar_add(
                            dst, acc, bqk_sb[:, 3 * qk + pp: 3 * qk + pp + 1],
                        )
                for tbl in range(4):
                    tb = 4 * tcw + tbl
                    accv = ps.tile([128, 384], FP32, tag="a", bufs=2)
                    for kc in range(NKC):
                        nc.tensor.matmul(
                            accv,
                            xt[:, kc * T + tb * 128: kc * T + (tb + 1) * 128],
                            wa[:, kc * WQKV + 768: kc * WQKV + WQKV],
                            start=(kc == 0),
                            stop=(kc == NKC - 1),
                        )
                    for pp in range(NPAIR):
                        vdst = vg[pp][tcw].rearrange("p (g c) -> p g c", c=65)
                        nc.vector.tensor_tensor(
                            out=vdst[:, 2 * tbl:2 * tbl + 2, 0:64],
                            in0=accv[:, pp * 128:(pp + 1) * 128]
                                .rearrange("p (h c) -> p h c", c=64),
                            in1=bvb_sb[:, pp * 128:(pp + 1) * 128]
                                .rearrange("p (h c) -> p h c", c=64),
                            op=mybir.AluOpType.add,
                        )

                # ---- attention for this t chunk, per head pair ----
                for pp in range(NPAIR):
                    nsb = 4 * tcw + 4  # causal: s blocks 0 .. 4*tcw+3
                    yps = [ps.tile([65, TCW], FP32, tag="yt", bufs=2, name=f"yps{tcw}_{pp}_{e}")
                           for e in range(2)]
                    for g in range(nsb // 2):
                        for e in range(2):
                            pt2 = ps.tile([128, 2 * TCW], FP32, tag="pt", bufs=2)
                            for j in range(2):
                                sbi = 2 * g + j
                                tcv, sbl = sbi // 4, sbi % 4
                                nc.tensor.matmul(
                                    pt2[:, j * TCW:(j + 1) * TCW],
                                    kT[pp][tcv][e * 64:(e + 1) * 64,
                                                sbl * 128:(sbl + 1) * 128],
                                    qT[pp][tcw][e * 64:(e + 1) * 64, :],
                                    start=True,
                                    stop=True,
                                )
                            ptsb = work.tile([128, 2 * TCW], BF16, tag="ptsb", bufs=4)
                            nc.scalar.activation(
                                ptsb, pt2, mybir.ActivationFunctionType.Exp,
                                scale=0.125,
                            )
                            for j in range(2):
                                sbi = 2 * g + j
                                psl = ptsb[:, j * TCW:(j + 1) * TCW]
                                if sbi >= 4 * tcw:  # diagonal-band block
                                    nc.gpsimd.affine_select(
                                        out=psl, in_=psl,
                                        compare_op=mybir.AluOpType.is_ge,
                                        fill=0.0,
                                        base=tcw * TCW - sbi * 128,
                                        channel_multiplier=-1,
                                        pattern=[[1, TCW]],
                                    )
                                nc.tensor.matmul(
                                    yps[e],
                                    vg[pp][sbi // 4][:, (2 * (sbi % 4) + e) * 65:
                                                     (2 * (sbi % 4) + e + 1) * 65],
                                    psl,
                                    start=(sbi == 0),
                                    stop=(sbi == nsb - 1),
                                )
                    for e in range(2):
                        dsb = work.tile([1, TCW], FP32, tag="dsb", bufs=2)
                        nc.vector.tensor_copy(dsb, yps[e][64:65, :])
                        rt = work.tile([1, TCW], FP32, tag="rt", bufs=2)
                        nc.vector.reciprocal_approx_fast(rt, dsb)
                        rbc = work.tile([64, TCW], FP32, tag="rbc", bufs=2)
                        nc.gpsimd.partition_broadcast(rbc, rt)
                        with nc.allow_low_precision(reason="bf16 attn out"):
                            nc.vector.tensor_tensor(
                                out=yT[pp][tcw][e * 64:(e + 1) * 64, :],
                                in0=yps[e][0:64, :],
                                in1=rbc,
                                op=mybir.AluOpType.mult,
                            )

                # ---- projection for this t chunk ----
                for tbl in range(4):
                    tb = 4 * tcw + tbl
                    osb = work.tile([128, C], FP32, tag="osb", bufs=2)
                    for ncw in range(2):
                        acc = ps.tile([128, 384], FP32, tag="a", bufs=2)
                        for cc in range(NPAIR):
                            nc.tensor.matmul(
                                acc,
                                yT[cc][tcw][:, tbl * 128:(tbl + 1) * 128],
                                wp[:, cc * C + ncw * 384: cc * C + (ncw + 1) * 384],
                                start=(cc == 0),
                                stop=(cc == NPAIR - 1),
                            )
                        nc.vector.tensor_copy(osb[:, ncw * 384:(ncw + 1) * 384], acc)
                    nc.sync.dma_start(
                        out=part[tb * 128:(tb + 1) * 128, :], in_=osb
                    )

    nc.compile()
    return nc


def _get_nc():
    global _BUILT
    if _BUILT is None:
        _BUILT = _build_nc()
    return _BUILT


def _build_in_maps(x, W_attn, b_attn, W_proj):
    in_maps = []
    for c in range(NCORES):
        b = c // 2
        hs = (c % 2) * HPC
        q0, k0, v0 = hs * D, C + hs * D, 2 * C + hs * D
        w = HPC * D  # 384
        xT_b = np.ascontiguousarray(x[b].T)
        wqkv_c = np.ascontiguousarray(
            np.concatenate(
                [W_attn[:, q0:q0 + w], W_attn[:, k0:k0 + w], W_attn[:, v0:v0 + w]],
                axis=1,
            )
        )
        bqk_c = np.stack(
            [b_attn[q0 + pp * 128: q0 + (pp + 1) * 128] for pp in range(3)]
            + [b_attn[k0 + pp * 128: k0 + (pp + 1) * 128] for pp in range(3)],
            axis=1,
        ).astype(np.float32)
        bvb_c = np.ascontiguousarray(
            np.broadcast_to(b_attn[v0:v0 + w][None, :], (128, w))
        ).astype(np.float32)
        wproj_c = np.ascontiguousarray(
            W_proj[hs * D: hs * D + w, :]
        ).astype(ml_dtypes.bfloat16)
        in_maps.append(
            {
                "xT": xT_b,
                "wqkv": wqkv_c,
                "bqk": bqk_c,
                "bvb": bvb_c,
                "wproj": wproj_c,
            }
        )
    return in_maps


def kernel(x, W_attn, b_attn, W_proj, b_proj):
    x = np.asarray(x, dtype=np.float32)
    W_attn = np.asarray(W_attn, dtype=np.float32)
    b_attn = np.asarray(b_attn, dtype=np.float32)
    W_proj = np.asarray(W_proj, dtype=np.float32)
    b_proj = np.asarray(b_proj, dtype=np.float32)

    nc = _get_nc()
    in_maps = _build_in_maps(x, W_attn, b_attn, W_proj)

    res = run_bass_kernel_spmd(nc, in_maps, core_ids=list(range(NCORES)))
    out = np.empty((B, T, C), dtype=np.float32)
    for b in range(B):
        out[b] = (
            res.results[2 * b]["part"]
            + res.results[2 * b + 1]["part"]
            + b_proj[None, :]
        )
    return out


# revision 9
# speedup vs baseline: 10535.9341x; 1.2918x over previous
"""Causal self-attention Bass/Tile kernel for Trainium2, SPMD over 8 NeuronCores.

Problem: B=4, T=2048, C=768, NH=12 heads, D=64. y = softmax(mask(qk^T/sqrt(D))) v,
with qkv = x@W_attn + b_attn and out = y@W_proj + b_proj.

Sharding: core c handles batch b = c//2 and heads [hs, hs+6) where hs = (c%2)*6
(data parallel over batch x tensor parallel over head-halves). Each core computes
a partial output part_c = y_c @ W_proj[rows of its heads]; the host sums the two
partials of each batch pair and adds b_proj (linear ops, exact in fp32).

Dataflow: one fused pipeline over t-chunks (tcw of 512). Per chunk: qkv
projection (fp32r), scores kT^T qT row-tiled per head pair (fp32r), exp on ACT
(scale=1/sqrt(D)) to bf16, causal mask via gpsimd affine_select on diagonal
blocks, PV accumulation in PSUM with a ones column per head so the softmax
denominator falls out of the matmul, fast-approx reciprocal + partition
broadcast + multiply for the normalization, then the projection matmul with
bf16 weights DMA'd straight from PSUM to DRAM. Tensors are split per
(pair, t-chunk) so the Tile scheduler can overlap phases.
"""

import os
import sys

for _p in ("/opt/trn_rl_repo", "/root/.axon_site/_ro/trn_rl_repo"):
    if os.path.isdir(_p) and _p not in sys.path:
        sys.path.insert(0, _p)
        break

import numpy as np
import ml_dtypes

import concourse.bass as bass  # noqa: F401
import concourse.mybir as mybir
import concourse.tile as tile
from concourse import bacc
from concourse.bass_utils import run_bass_kernel_spmd

FP32 = mybir.dt.float32
FP32R = mybir.dt.float32r
BF16 = mybir.dt.bfloat16

B, T, C = 4, 2048, 768
NH, D = 12, 64
NCORES = 8
NKC = C // 128          # 6 contraction chunks for qkv
TCW = 512
NTC = T // TCW          # 4 t chunks
HPC = 6                 # heads per core
NPAIR = 3               # head pairs per core
WQKV = 3 * HPC * D      # 1152

_BUILT = None


def _build_nc():
    nc = bacc.Bacc("TRN2", target_bir_lowering=False, debug=False, num_devices=NCORES)

    xT = nc.dram_tensor("xT", [C, T], BF16, kind="ExternalInput")
    wqkv = nc.dram_tensor("wqkv", [C, WQKV], BF16, kind="ExternalInput")
    bqk = nc.dram_tensor("bqk", [128, 6], FP32, kind="ExternalInput")
    bvb = nc.dram_tensor("bvb", [128, 384], FP32, kind="ExternalInput")
    wproj = nc.dram_tensor("wproj", [384, C], BF16, kind="ExternalInput")
    part = nc.dram_tensor("part", [T, C], FP32, kind="ExternalOutput")

    with tile.TileContext(nc) as tc:
        with tc.sbuf_pool(name="pers", bufs=1) as pers, \
             tc.sbuf_pool(name="work", bufs=1) as work, \
             tc.psum_pool(name="ps", bufs=1) as ps:
            xt = pers.tile([128, NKC * T], BF16)
            wa = pers.tile([128, NKC * WQKV], BF16)
            wp = pers.tile([128, 3 * C], BF16)
            bqk_sb = pers.tile([128, 6], FP32)
            bvb_sb = pers.tile([128, 384], FP32)
            qT = [[pers.tile([128, TCW], BF16, tag=f"qT{p}_{t}", name=f"qT{p}_{t}")
                   for t in range(NTC)] for p in range(NPAIR)]
            kT = [[pers.tile([128, TCW], BF16, tag=f"kT{p}_{t}", name=f"kT{p}_{t}")
                   for t in range(NTC)] for p in range(NPAIR)]
            vg = [[pers.tile([128, 8 * 65], BF16, tag=f"vg{p}_{t}", name=f"vg{p}_{t}")
                   for t in range(NTC)] for p in range(NPAIR)]
            yT = [[pers.tile([128, TCW], BF16, tag=f"yT{p}_{t}", name=f"yT{p}_{t}")
                   for t in range(NTC)] for p in range(NPAIR)]

            nc.sync.dma_start(
                out=wp.rearrange("p (k c) -> p k c", c=C),
                in_=wproj.rearrange("(k p) c -> p k c", p=128),
            )
            nc.sync.dma_start(out=bqk_sb, in_=bqk[:, :])
            nc.sync.dma_start(out=bvb_sb, in_=bvb[:, :])
            for kc in range(NKC):
                nc.sync.dma_start(
                    out=wa[:, kc * WQKV:(kc + 1) * WQKV],
                    in_=wqkv[kc * 128:(kc + 1) * 128, :],
                )
            for tcw in range(NTC):
                for kc in range(NKC):
                    nc.sync.dma_start(
                        out=xt[:, kc * T + tcw * TCW: kc * T + (tcw + 1) * TCW],
                        in_=xT[kc * 128:(kc + 1) * 128, tcw * TCW:(tcw + 1) * TCW],
                    )
            for p in range(NPAIR):
                for t in range(NTC):
                    ones_cols = vg[p][t].rearrange("p (g c) -> p g c", c=65)[:, :, 64:65]
                    nc.vector.memset(ones_cols, 1.0)

            def emit_a(tcw):
                # qkv projection for t chunk tcw
                for pp in range(NPAIR):
                    for qk in range(2):  # 0 -> q, 1 -> k
                        dst = (qT if qk == 0 else kT)[pp][tcw]
                        acc = ps.tile([128, TCW], FP32, tag="a", bufs=2,
                                      name=f"qk{tcw}_{pp}_{qk}")
                        for kc in range(NKC):
                            co = kc * WQKV + qk * 384 + pp * 128
                            nc.tensor.matmul(
                                acc,
                                wa[:, co:co + 128],
                                xt[:, kc * T + tcw * TCW: kc * T + (tcw + 1) * TCW],
                                start=(kc == 0),
                                stop=(kc == NKC - 1),
                            )
                        nc.vector.tensor_scalar_add(
                            dst, acc, bqk_sb[:, 3 * qk + pp: 3 * qk + pp + 1],
                        )
                for tbl in range(4):
                    tb = 4 * tcw + tbl
                    accv = ps.tile([128, 384], FP32, tag="a", bufs=2,
                                   name=f"v{tcw}_{tbl}")
                    for kc in range(NKC):
                        nc.tensor.matmul(
                            accv,
                            xt[:, kc * T + tb * 128: kc * T + (tb + 1) * 128],
                            wa[:, kc * WQKV + 768: kc * WQKV + WQKV],
                            start=(kc == 0),
                            stop=(kc == NKC - 1),
                        )
                    for pp in range(NPAIR):
                        vdst = vg[pp][tcw].rearrange("p (g c) -> p g c", c=65)
                        nc.vector.tensor_tensor(
                            out=vdst[:, 2 * tbl:2 * tbl + 2, 0:64],
                            in0=accv[:, pp * 128:(pp + 1) * 128]
                                .rearrange("p (h c) -> p h c", c=64),
                            in1=bvb_sb[:, pp * 128:(pp + 1) * 128]
                                .rearrange("p (h c) -> p h c", c=64),
                            op=mybir.AluOpType.add,
                        )

            def emit_b(tcw):
                # attention for t chunk tcw, per head pair
                for pp in range(NPAIR):
                    nsb = 4 * tcw + 4  # causal: s blocks 0 .. 4*tcw+3
                    yps = [ps.tile([65, TCW], FP32, tag="yt", bufs=2,
                                   name=f"yps{tcw}_{pp}_{e}")
                           for e in range(2)]
                    for g in range(nsb // 2):
                        for e in range(2):
                            pt2 = ps.tile([128, 2 * TCW], FP32, tag="pt", bufs=2,
                                          name=f"pt{tcw}_{pp}_{g}_{e}")
                            for j in range(2):
                                sbi = 2 * g + j
                                tcv, sbl = sbi // 4, sbi % 4
                                nc.tensor.matmul(
                                    pt2[:, j * TCW:(j + 1) * TCW],
                                    kT[pp][tcv][e * 64:(e + 1) * 64,
                                                sbl * 128:(sbl + 1) * 128],
                                    qT[pp][tcw][e * 64:(e + 1) * 64, :],
                                    start=True,
                                    stop=True,
                                )
                            ptsb = work.tile([128, 2 * TCW], BF16, tag="ptsb",
                                             bufs=4, name=f"ptsb{tcw}_{pp}_{g}_{e}")
                            nc.scalar.activation(
                                ptsb, pt2, mybir.ActivationFunctionType.Exp,
                                scale=0.125,
                            )
                            for j in range(2):
                                sbi = 2 * g + j
                                psl = ptsb[:, j * TCW:(j + 1) * TCW]
                                if sbi >= 4 * tcw:  # diagonal-band block
                                    nc.gpsimd.affine_select(
                                        out=psl, in_=psl,
                                        compare_op=mybir.AluOpType.is_ge,
                                        fill=0.0,
                                        base=tcw * TCW - sbi * 128,
                                        channel_multiplier=-1,
                                        pattern=[[1, TCW]],
                                    )
                                nc.tensor.matmul(
                                    yps[e],
                                    vg[pp][sbi // 4][:, (2 * (sbi % 4) + e) * 65:
                                                     (2 * (sbi % 4) + e + 1) * 65],
                                    psl,
                                    start=(sbi == 0),
                                    stop=(sbi == nsb - 1),
                                )
                    for e in range(2):
                        dsb = work.tile([1, TCW], FP32, tag="dsb", bufs=2,
                                        name=f"dsb{tcw}_{pp}_{e}")
                        nc.vector.tensor_copy(dsb, yps[e][64:65, :])
                        rt = work.tile([1, TCW], FP32, tag="rt", bufs=2,
                                       name=f"rt{tcw}_{pp}_{e}")
                        nc.vector.reciprocal_approx_fast(rt, dsb)
                        rbc = work.tile([64, TCW], FP32, tag="rbc", bufs=2,
                                        name=f"rbc{tcw}_{pp}_{e}")
                        nc.gpsimd.partition_broadcast(rbc, rt)
                        with nc.allow_low_precision(reason="bf16 attn out"):
                            nc.vector.tensor_tensor(
                                out=yT[pp][tcw][e * 64:(e + 1) * 64, :],
                                in0=yps[e][0:64, :],
                                in1=rbc,
                                op=mybir.AluOpType.mult,
                            )

            def emit_c(tcw):
                # projection for t chunk tcw
                for tbl in range(4):
                    tb = 4 * tcw + tbl
                    osb = work.tile([128, C], FP32, tag="osb", bufs=2,
                                    name=f"osb{tcw}_{tbl}")
                    for ncw in range(2):
                        acc = ps.tile([128, 384], FP32, tag="a", bufs=2,
                                      name=f"c{tcw}_{tbl}_{ncw}")
                        for cc in range(NPAIR):
                            nc.tensor.matmul(
                                acc,
                                yT[cc][tcw][:, tbl * 128:(tbl + 1) * 128],
                                wp[:, cc * C + ncw * 384: cc * C + (ncw + 1) * 384],
                                start=(cc == 0),
                                stop=(cc == NPAIR - 1),
                            )
                        nc.vector.tensor_copy(osb[:, ncw * 384:(ncw + 1) * 384], acc)
                    nc.sync.dma_start(
                        out=part[tb * 128:(tb + 1) * 128, :], in_=osb
                    )

            emit_a(0)
            for tcw in range(NTC):
                emit_b(tcw)
                if tcw + 1 < NTC:
                    emit_a(tcw + 1)
                emit_c(tcw)

    nc.compile()
    return nc


def _get_nc():
    global _BUILT
    if _BUILT is None:
        _BUILT = _build_nc()
    return _BUILT


def _build_in_maps(x, W_attn, b_attn, W_proj):
    in_maps = []
    for c in range(NCORES):
        b = c // 2
        hs = (c % 2) * HPC
        q0, k0, v0 = hs * D, C + hs * D, 2 * C + hs * D
        w = HPC * D  # 384
        xT_b = np.ascontiguousarray(x[b].T).astype(ml_dtypes.bfloat16)
        wqkv_c = np.ascontiguousarray(
            np.concatenate(
                [W_attn[:, q0:q0 + w], W_attn[:, k0:k0 + w], W_attn[:, v0:v0 + w]],
                axis=1,
            )
        ).astype(ml_dtypes.bfloat16)
        bqk_c = np.stack(
            [b_attn[q0 + pp * 128: q0 + (pp + 1) * 128] for pp in range(3)]
            + [b_attn[k0 + pp * 128: k0 + (pp + 1) * 128] for pp in range(3)],
            axis=1,
        ).astype(np.float32)
        bvb_c = np.ascontiguousarray(
            np.broadcast_to(b_attn[v0:v0 + w][None, :], (128, w))
        ).astype(np.float32)
        wproj_c = np.ascontiguousarray(
            W_proj[hs * D: hs * D + w, :]
        ).astype(ml_dtypes.bfloat16)
        in_maps.append(
            {
                "xT": xT_b,
                "wqkv": wqkv_c,
                "bqk": bqk_c,
                "bvb": bvb_c,
                "wproj": wproj_c,
            }
        )
    return in_maps


def kernel(x, W_attn, b_attn, W_proj, b_proj):
    x = np.asarray(x, dtype=np.float32)
    W_attn = np.asarray(W_attn, dtype=np.float32)
    b_attn = np.asarray(b_attn, dtype=np.float32)
    W_proj = np.asarray(W_proj, dtype=np.float32)
    b_proj = np.asarray(b_proj, dtype=np.float32)

    nc = _get_nc()
    in_maps = _build_in_maps(x, W_attn, b_attn, W_proj)

    res = run_bass_kernel_spmd(nc, in_maps, core_ids=list(range(NCORES)))
    out = np.empty((B, T, C), dtype=np.float32)
    for b in range(B):
        out[b] = (
            res.results[2 * b]["part"]
            + res.results[2 * b + 1]["part"]
            + b_proj[None, :]
        )
    return out


# revision 11
# speedup vs baseline: 10657.6008x; 1.0115x over previous
"""Causal self-attention Bass/Tile kernel for Trainium2, SPMD over 8 NeuronCores.

Problem: B=4, T=2048, C=768, NH=12 heads, D=64. y = softmax(mask(qk^T/sqrt(D))) v,
with qkv = x@W_attn + b_attn and out = y@W_proj + b_proj.

Sharding: core c handles batch b = c//2 and heads [hs, hs+6) where hs = (c%2)*6
(data parallel over batch x tensor parallel over head-halves). Each core computes
a partial output part_c = y_c @ W_proj[rows of its heads]; the host sums the two
partials of each batch pair and adds b_proj (linear ops, exact in fp32).

Dataflow: one fused pipeline over t-chunks (tcw of 512). Per chunk: qkv
projection (fp32r), scores kT^T qT row-tiled per head pair (fp32r), exp on ACT
(scale=1/sqrt(D)) to bf16, causal mask via gpsimd affine_select on diagonal
blocks, PV accumulation in PSUM with a ones column per head so the softmax
denominator falls out of the matmul, fast-approx reciprocal + partition
broadcast + multiply for the normalization, then the projection matmul with
bf16 weights DMA'd straight from PSUM to DRAM. Tensors are split per
(pair, t-chunk) so the Tile scheduler can overlap phases.
"""

import os
import sys

for _p in ("/opt/trn_rl_repo", "/root/.axon_site/_ro/trn_rl_repo"):
    if os.path.isdir(_p) and _p not in sys.path:
        sys.path.insert(0, _p)
        break

import numpy as np
import ml_dtypes

import concourse.bass as bass  # noqa: F401
import concourse.mybir as mybir
import concourse.tile as tile
from concourse import bacc
from concourse.bass_utils import run_bass_kernel_spmd

FP32 = mybir.dt.float32
FP32R = mybir.dt.float32r
BF16 = mybir.dt.bfloat16

B, T, C = 4, 2048, 768
NH, D = 12, 64
NCORES = 8
NKC = C // 128          # 6 contraction chunks for qkv
TCW = 512
NTC = T // TCW          # 4 t chunks
HPC = 6                 # heads per core
NPAIR = 3               # head pairs per core
WQKV = 3 * HPC * D      # 1152

_BUILT = None


def _build_nc():
    nc = bacc.Bacc("TRN2", target_bir_lowering=False, debug=False, num_devices=NCORES)

    xT = nc.dram_tensor("xT", [C, T], BF16, kind="ExternalInput")
    wqkv = nc.dram_tensor("wqkv", [C, WQKV], BF16, kind="ExternalInput")
    bqk = nc.dram_tensor("bqk", [128, 6], FP32, kind="ExternalInput")
    bvb = nc.dram_tensor("bvb", [128, 384], FP32, kind="ExternalInput")
    wproj = nc.dram_tensor("wproj", [384, C], BF16, kind="ExternalInput")
    part = nc.dram_tensor("part", [T, C], FP32, kind="ExternalOutput")

    with tile.TileContext(nc) as tc:
        with tc.sbuf_pool(name="pers", bufs=1) as pers, \
             tc.sbuf_pool(name="work", bufs=1) as work, \
             tc.psum_pool(name="ps", bufs=1) as ps:
            xts = [pers.tile([128, NKC * TCW], BF16, tag=f"xts{t}", name=f"xts{t}")
                   for t in range(NTC)]
            was = [pers.tile([128, WQKV], BF16, tag=f"was{k}", name=f"was{k}")
                   for k in range(NKC)]
            wp = pers.tile([128, 3 * C], BF16)
            bqk_sb = pers.tile([128, 6], FP32)
            bvb_sb = pers.tile([128, 384], FP32)
            qT = [[pers.tile([128, TCW], BF16, tag=f"qT{p}_{t}", name=f"qT{p}_{t}")
                   for t in range(NTC)] for p in range(NPAIR)]
            kT = [[pers.tile([128, TCW], BF16, tag=f"kT{p}_{t}", name=f"kT{p}_{t}")
                   for t in range(NTC)] for p in range(NPAIR)]
            vg = [[pers.tile([128, 8 * 65], BF16, tag=f"vg{p}_{t}", name=f"vg{p}_{t}")
                   for t in range(NTC)] for p in range(NPAIR)]
            yT = [[pers.tile([128, TCW], BF16, tag=f"yT{p}_{t}", name=f"yT{p}_{t}")
                   for t in range(NTC)] for p in range(NPAIR)]

            nc.sync.dma_start(out=bqk_sb, in_=bqk[:, :])
            nc.sync.dma_start(out=bvb_sb, in_=bvb[:, :])
            for kc in range(NKC):
                nc.sync.dma_start(
                    out=was[kc],
                    in_=wqkv[kc * 128:(kc + 1) * 128, :],
                )
                nc.sync.dma_start(
                    out=xts[0][:, kc * TCW:(kc + 1) * TCW],
                    in_=xT[kc * 128:(kc + 1) * 128, 0:TCW],
                )
            for tcw in range(1, NTC):
                for kc in range(NKC):
                    nc.sync.dma_start(
                        out=xts[tcw][:, kc * TCW:(kc + 1) * TCW],
                        in_=xT[kc * 128:(kc + 1) * 128, tcw * TCW:(tcw + 1) * TCW],
                    )
            nc.sync.dma_start(
                out=wp.rearrange("p (k c) -> p k c", c=C),
                in_=wproj.rearrange("(k p) c -> p k c", p=128),
            )
            for p in range(NPAIR):
                for t in range(NTC):
                    ones_cols = vg[p][t].rearrange("p (g c) -> p g c", c=65)[:, :, 64:65]
                    nc.vector.memset(ones_cols, 1.0)

            def emit_a(tcw):
                # qkv projection for t chunk tcw
                for pp in range(NPAIR):
                    for qk in range(2):  # 0 -> q, 1 -> k
                        dst = (qT if qk == 0 else kT)[pp][tcw]
                        acc = ps.tile([128, TCW], FP32, tag="a", bufs=2,
                                      name=f"qk{tcw}_{pp}_{qk}")
                        for kc in range(NKC):
                            co = qk * 384 + pp * 128
                            nc.tensor.matmul(
                                acc,
                                was[kc][:, co:co + 128],
                                xts[tcw][:, kc * TCW:(kc + 1) * TCW],
                                start=(kc == 0),
                                stop=(kc == NKC - 1),
                            )
                        nc.vector.tensor_scalar_add(
                            dst, acc, bqk_sb[:, 3 * qk + pp: 3 * qk + pp + 1],
                        )
                for tbl in range(4):
                    tb = 4 * tcw + tbl
                    accv = ps.tile([128, 384], FP32, tag="a", bufs=2,
                                   name=f"v{tcw}_{tbl}")
                    for kc in range(NKC):
                        nc.tensor.matmul(
                            accv,
                            xts[tcw][:, kc * TCW + tbl * 128:
                                      kc * TCW + (tbl + 1) * 128],
                            was[kc][:, 768:WQKV],
                            start=(kc == 0),
                            stop=(kc == NKC - 1),
                        )
                    for pp in range(NPAIR):
                        vdst = vg[pp][tcw].rearrange("p (g c) -> p g c", c=65)
                        nc.vector.tensor_tensor(
                            out=vdst[:, 2 * tbl:2 * tbl + 2, 0:64],
                            in0=accv[:, pp * 128:(pp + 1) * 128]
                                .rearrange("p (h c) -> p h c", c=64),
                            in1=bvb_sb[:, pp * 128:(pp + 1) * 128]
                                .rearrange("p (h c) -> p h c", c=64),
                            op=mybir.AluOpType.add,
                        )

            def emit_b(tcw):
                # attention for t chunk tcw, per head pair
                for pp in range(NPAIR):
                    nsb = 4 * tcw + 4  # causal: s blocks 0 .. 4*tcw+3
                    yps = [ps.tile([65, TCW], FP32, tag="yt", bufs=2,
                                   name=f"yps{tcw}_{pp}_{e}")
                           for e in range(2)]
                    for g in range(nsb // 2):
                        for e in range(2):
                            pt2 = ps.tile([128, 2 * TCW], FP32, tag="pt", bufs=2,
                                          name=f"pt{tcw}_{pp}_{g}_{e}")
                            for j in range(2):
                                sbi = 2 * g + j
                                tcv, sbl = sbi // 4, sbi % 4
                                nc.tensor.matmul(
                                    pt2[:, j * TCW:(j + 1) * TCW],
                                    kT[pp][tcv][e * 64:(e + 1) * 64,
                                                sbl * 128:(sbl + 1) * 128],
                                    qT[pp][tcw][e * 64:(e + 1) * 64, :],
                                    start=True,
                                    stop=True,
                                )
                            ptsb = work.tile([128, 2 * TCW], BF16, tag="ptsb",
                                             bufs=4, name=f"ptsb{tcw}_{pp}_{g}_{e}")
                            nc.scalar.activation(
                                ptsb, pt2, mybir.ActivationFunctionType.Exp,
                                scale=0.125,
                            )
                            for j in range(2):
                                sbi = 2 * g + j
                                off = max(0, (sbi - 4 * tcw) * 128)
                                psl = ptsb[:, j * TCW + off:(j + 1) * TCW]
                                if sbi >= 4 * tcw:  # diagonal-band block
                                    nc.gpsimd.affine_select(
                                        out=psl, in_=psl,
                                        compare_op=mybir.AluOpType.is_ge,
                                        fill=0.0,
                                        base=0,
                                        channel_multiplier=-1,
                                        pattern=[[1, TCW - off]],
                                    )
                                nc.tensor.matmul(
                                    yps[e][:, off:],
                                    vg[pp][sbi // 4][:, (2 * (sbi % 4) + e) * 65:
                                                     (2 * (sbi % 4) + e + 1) * 65],
                                    psl,
                                    start=(sbi == 0),
                                    stop=(sbi == nsb - 1),
                                )
                    for e in range(2):
                        dsb = work.tile([1, TCW], FP32, tag="dsb", bufs=2,
                                        name=f"dsb{tcw}_{pp}_{e}")
                        nc.vector.tensor_copy(dsb, yps[e][64:65, :])
                        rt = work.tile([1, TCW], FP32, tag="rt", bufs=2,
                                       name=f"rt{tcw}_{pp}_{e}")
                        nc.vector.reciprocal_approx_fast(rt, dsb)
                        rbc = work.tile([64, TCW], FP32, tag="rbc", bufs=2,
                                        name=f"rbc{tcw}_{pp}_{e}")
                        nc.gpsimd.partition_broadcast(rbc, rt)
                        with nc.allow_low_precision(reason="bf16 attn out"):
                            nc.vector.tensor_tensor(
                                out=yT[pp][tcw][e * 64:(e + 1) * 64, :],
                                in0=yps[e][0:64, :],
                                in1=rbc,
                                op=mybir.AluOpType.mult,
                            )

            def emit_c(tcw):
                # projection for t chunk tcw
                for tbl in range(4):
                    tb = 4 * tcw + tbl
                    osb = work.tile([128, C], FP32, tag="osb", bufs=2,
                                    name=f"osb{tcw}_{tbl}")
                    for ncw in range(2):
                        acc = ps.tile([128, 384], FP32, tag="a", bufs=2,
                                      name=f"c{tcw}_{tbl}_{ncw}")
                        for cc in range(NPAIR):
                            nc.tensor.matmul(
                                acc,
                                yT[cc][tcw][:, tbl * 128:(tbl + 1) * 128],
                                wp[:, cc * C + ncw * 384: cc * C + (ncw + 1) * 384],
                                start=(cc == 0),
                                stop=(cc == NPAIR - 1),
                            )
                        nc.vector.tensor_copy(osb[:, ncw * 384:(ncw + 1) * 384], acc)
                    nc.sync.dma_start(
                        out=part[tb * 128:(tb + 1) * 128, :], in_=osb
                    )

            emit_a(0)
            for tcw in range(NTC):
                emit_b(tcw)
                if tcw + 1 < NTC:
                    emit_a(tcw + 1)
                emit_c(tcw)

    nc.compile()
    return nc


def _get_nc():
    global _BUILT
    if _BUILT is None:
        _BUILT = _build_nc()
    return _BUILT


def _build_in_maps(x, W_attn, b_attn, W_proj):
    in_maps = []
    for c in range(NCORES):
        b = c // 2
        hs = (c % 2) * HPC
        q0, k0, v0 = hs * D, C + hs * D, 2 * C + hs * D
        w = HPC * D  # 384
        xT_b = np.ascontiguousarray(x[b].T).astype(ml_dtypes.bfloat16)
        wqkv_c = np.ascontiguousarray(
            np.concatenate(
                [W_attn[:, q0:q0 + w], W_attn[:, k0:k0 + w], W_attn[:, v0:v0 + w]],
                axis=1,
            )
        ).astype(ml_dtypes.bfloat16)
        bqk_c = np.stack(
            [b_attn[q0 + pp * 128: q0 + (pp + 1) * 128] for pp in range(3)]
            + [b_attn[k0 + pp * 128: k0 + (pp + 1) * 128] for pp in range(3)],
            axis=1,
        ).astype(np.float32)
        bvb_c = np.ascontiguousarray(
            np.broadcast_to(b_attn[v0:v0 + w][None, :], (128, w))
        ).astype(np.float32)
        wproj_c = np.ascontiguousarray(
            W_proj[hs * D: hs * D + w, :]
        ).astype(ml_dtypes.bfloat16)
        in_maps.append(
            {
                "xT": xT_b,
                "wqkv": wqkv_c,
                "bqk": bqk_c,
                "bvb": bvb_c,
                "wproj": wproj_c,
            }
        )
    return in_maps


def kernel(x, W_attn, b_attn, W_proj, b_proj):
    x = np.asarray(x, dtype=np.float32)
    W_attn = np.asarray(W_attn, dtype=np.float32)
    b_attn = np.asarray(b_attn, dtype=np.float32)
    W_proj = np.asarray(W_proj, dtype=np.float32)
    b_proj = np.asarray(b_proj, dtype=np.float32)

    nc = _get_nc()
    in_maps = _build_in_maps(x, W_attn, b_attn, W_proj)

    res = run_bass_kernel_spmd(nc, in_maps, core_ids=list(range(NCORES)))
    out = np.empty((B, T, C), dtype=np.float32)
    for b in range(B):
        out[b] = (
            res.results[2 * b]["part"]
            + res.results[2 * b + 1]["part"]
            + b_proj[None, :]
        )
    return out


# revision 12
# speedup vs baseline: 11011.7628x; 1.0332x over previous
"""Causal self-attention Bass/Tile kernel for Trainium2, SPMD over 8 NeuronCores.

Problem: B=4, T=2048, C=768, NH=12 heads, D=64. y = softmax(mask(qk^T/sqrt(D))) v,
with qkv = x@W_attn + b_attn and out = y@W_proj + b_proj.

Sharding: core c handles batch b = c//2 and heads [hs, hs+6) where hs = (c%2)*6
(data parallel over batch x tensor parallel over head-halves). Each core computes
a partial output part_c = y_c @ W_proj[rows of its heads]; the host sums the two
partials of each batch pair and adds b_proj (linear ops, exact in fp32).

Dataflow: one fused pipeline over t-chunks (tcw of 512). Per chunk: qkv
projection (fp32r), scores kT^T qT row-tiled per head pair (fp32r), exp on ACT
(scale=1/sqrt(D)) to bf16, causal mask via gpsimd affine_select on diagonal
blocks, PV accumulation in PSUM with a ones column per head so the softmax
denominator falls out of the matmul, fast-approx reciprocal + partition
broadcast + multiply for the normalization, then the projection matmul with
bf16 weights DMA'd straight from PSUM to DRAM. Tensors are split per
(pair, t-chunk) so the Tile scheduler can overlap phases.
"""

import os
import sys

for _p in ("/opt/trn_rl_repo", "/root/.axon_site/_ro/trn_rl_repo"):
    if os.path.isdir(_p) and _p not in sys.path:
        sys.path.insert(0, _p)
        break

import numpy as np
import ml_dtypes

import concourse.bass as bass  # noqa: F401
import concourse.mybir as mybir
import concourse.tile as tile
from concourse import bacc
from concourse.bass_utils import run_bass_kernel_spmd

FP32 = mybir.dt.float32
FP32R = mybir.dt.float32r
BF16 = mybir.dt.bfloat16

B, T, C = 4, 2048, 768
NH, D = 12, 64
NCORES = 8
NKC = C // 128          # 6 contraction chunks for qkv
TCW = 512
NTC = T // TCW          # 4 t chunks
HPC = 6                 # heads per core
NPAIR = 3               # head pairs per core
WQKV = 3 * HPC * D      # 1152

_BUILT = None


def _build_nc():
    nc = bacc.Bacc("TRN2", target_bir_lowering=False, debug=False, num_devices=NCORES)

    xT = nc.dram_tensor("xT", [C, T], BF16, kind="ExternalInput")
    wqkv = nc.dram_tensor("wqkv", [C, WQKV], BF16, kind="ExternalInput")
    bqk = nc.dram_tensor("bqk", [128, 6], FP32, kind="ExternalInput")
    bvb = nc.dram_tensor("bvb", [128, 384], FP32, kind="ExternalInput")
    wproj = nc.dram_tensor("wproj", [384, C], BF16, kind="ExternalInput")
    part = nc.dram_tensor("part", [T, C], FP32, kind="ExternalOutput")

    with tile.TileContext(nc) as tc:
        with tc.sbuf_pool(name="pers", bufs=1) as pers, \
             tc.sbuf_pool(name="work", bufs=1) as work, \
             tc.psum_pool(name="ps", bufs=1) as ps:
            xts = [pers.tile([128, NKC * TCW], BF16, tag=f"xts{t}", name=f"xts{t}")
                   for t in range(NTC)]
            was = [pers.tile([128, WQKV], BF16, tag=f"was{k}", name=f"was{k}")
                   for k in range(NKC)]
            wp = pers.tile([128, 3 * C], BF16)
            bqk_sb = pers.tile([128, 6], FP32)
            bvb_sb = pers.tile([128, 384], FP32)
            qT = [[pers.tile([128, TCW], BF16, tag=f"qT{p}_{t}", name=f"qT{p}_{t}")
                   for t in range(NTC)] for p in range(NPAIR)]
            kT = [[pers.tile([128, TCW], BF16, tag=f"kT{p}_{t}", name=f"kT{p}_{t}")
                   for t in range(NTC)] for p in range(NPAIR)]
            vg = [[pers.tile([128, 8 * 65], BF16, tag=f"vg{p}_{t}", name=f"vg{p}_{t}")
                   for t in range(NTC)] for p in range(NPAIR)]
            yT = [[pers.tile([128, TCW], BF16, tag=f"yT{p}_{t}", name=f"yT{p}_{t}")
                   for t in range(NTC)] for p in range(NPAIR)]

            nc.sync.dma_start(out=bqk_sb, in_=bqk[:, :])
            nc.sync.dma_start(out=bvb_sb, in_=bvb[:, :])
            for kc in range(NKC):
                nc.sync.dma_start(
                    out=was[kc],
                    in_=wqkv[kc * 128:(kc + 1) * 128, :],
                )
                nc.sync.dma_start(
                    out=xts[0][:, kc * TCW:(kc + 1) * TCW],
                    in_=xT[kc * 128:(kc + 1) * 128, 0:TCW],
                )
            for tcw in range(1, NTC):
                for kc in range(NKC):
                    nc.sync.dma_start(
                        out=xts[tcw][:, kc * TCW:(kc + 1) * TCW],
                        in_=xT[kc * 128:(kc + 1) * 128, tcw * TCW:(tcw + 1) * TCW],
                    )
            nc.sync.dma_start(
                out=wp.rearrange("p (k c) -> p k c", c=C),
                in_=wproj.rearrange("(k p) c -> p k c", p=128),
            )
            for p in range(NPAIR):
                for t in range(NTC):
                    ones_cols = vg[p][t].rearrange("p (g c) -> p g c", c=65)[:, :, 64:65]
                    nc.vector.memset(ones_cols, 1.0)

            def emit_a(tcw):
                # qkv projection for t chunk tcw
                for pp in range(NPAIR):
                    for qk in range(2):  # 0 -> q, 1 -> k
                        dst = (qT if qk == 0 else kT)[pp][tcw]
                        acc = ps.tile([128, TCW], FP32, tag="a", bufs=2,
                                      name=f"qk{tcw}_{pp}_{qk}")
                        for kc in range(NKC):
                            co = qk * 384 + pp * 128
                            nc.tensor.matmul(
                                acc,
                                was[kc][:, co:co + 128],
                                xts[tcw][:, kc * TCW:(kc + 1) * TCW],
                                start=(kc == 0),
                                stop=(kc == NKC - 1),
                            )
                        nc.vector.tensor_scalar_add(
                            dst, acc, bqk_sb[:, 3 * qk + pp: 3 * qk + pp + 1],
                        )
                for tbl in range(4):
                    tb = 4 * tcw + tbl
                    accv = ps.tile([128, 384], FP32, tag="a", bufs=2,
                                   name=f"v{tcw}_{tbl}")
                    for kc in range(NKC):
                        nc.tensor.matmul(
                            accv,
                            xts[tcw][:, kc * TCW + tbl * 128:
                                      kc * TCW + (tbl + 1) * 128],
                            was[kc][:, 768:WQKV],
                            start=(kc == 0),
                            stop=(kc == NKC - 1),
                        )
                    for pp in range(NPAIR):
                        vdst = vg[pp][tcw].rearrange("p (g c) -> p g c", c=65)
                        nc.vector.tensor_tensor(
                            out=vdst[:, 2 * tbl:2 * tbl + 2, 0:64],
                            in0=accv[:, pp * 128:(pp + 1) * 128]
                                .rearrange("p (h c) -> p h c", c=64),
                            in1=bvb_sb[:, pp * 128:(pp + 1) * 128]
                                .rearrange("p (h c) -> p h c", c=64),
                            op=mybir.AluOpType.add,
                        )

            def emit_b(tcw):
                # attention for t chunk tcw, per head pair
                for pp in range(NPAIR):
                    nsb = 4 * tcw + 4  # causal: s blocks 0 .. 4*tcw+3
                    yps = [ps.tile([65, TCW], FP32, tag="yt", bufs=2,
                                   name=f"yps{tcw}_{pp}_{e}")
                           for e in range(2)]
                    for g in range(nsb // 2):
                        # one psum/sbuf tile per s-block j, holding both heads;
                        # scores alternate row groups (e) so the PE runs the
                        # 64-contraction matmuls concurrently
                        ptj = [ps.tile([128, 2 * TCW], FP32, tag="pt", bufs=2,
                                       name=f"pt{tcw}_{pp}_{g}_{j}")
                               for j in range(2)]
                        for j in range(2):
                            sbi = 2 * g + j
                            tcv, sbl = sbi // 4, sbi % 4
                            for e in range(2):
                                nc.tensor.matmul(
                                    ptj[j][:, e * TCW:(e + 1) * TCW],
                                    kT[pp][tcv][e * 64:(e + 1) * 64,
                                                sbl * 128:(sbl + 1) * 128],
                                    qT[pp][tcw][e * 64:(e + 1) * 64, :],
                                    start=True,
                                    stop=True,
                                )
                        for j in range(2):
                            sbi = 2 * g + j
                            off = max(0, (sbi - 4 * tcw) * 128)
                            ptsb = work.tile([128, 2 * TCW], BF16, tag="ptsb",
                                             bufs=4, name=f"ptsb{tcw}_{pp}_{g}_{j}")
                            nc.scalar.activation(
                                ptsb, ptj[j], mybir.ActivationFunctionType.Exp,
                                scale=0.125,
                            )
                            for e in range(2):
                                psl = ptsb[:, e * TCW + off:(e + 1) * TCW]
                                if sbi >= 4 * tcw:  # diagonal-band block
                                    nc.gpsimd.affine_select(
                                        out=psl, in_=psl,
                                        compare_op=mybir.AluOpType.is_ge,
                                        fill=0.0,
                                        base=0,
                                        channel_multiplier=-1,
                                        pattern=[[1, TCW - off]],
                                    )
                                nc.tensor.matmul(
                                    yps[e][:, off:],
                                    vg[pp][sbi // 4][:, (2 * (sbi % 4) + e) * 65:
                                                     (2 * (sbi % 4) + e + 1) * 65],
                                    psl,
                                    start=(sbi == 0),
                                    stop=(sbi == nsb - 1),
                                )
                    for e in range(2):
                        dsb = work.tile([1, TCW], FP32, tag="dsb", bufs=2,
                                        name=f"dsb{tcw}_{pp}_{e}")
                        nc.vector.tensor_copy(dsb, yps[e][64:65, :])
                        rt = work.tile([1, TCW], FP32, tag="rt", bufs=2,
                                       name=f"rt{tcw}_{pp}_{e}")
                        nc.vector.reciprocal_approx_fast(rt, dsb)
                        rbc = work.tile([64, TCW], FP32, tag="rbc", bufs=2,
                                        name=f"rbc{tcw}_{pp}_{e}")
                        nc.gpsimd.partition_broadcast(rbc, rt)
                        with nc.allow_low_precision(reason="bf16 attn out"):
                            nc.vector.tensor_tensor(
                                out=yT[pp][tcw][e * 64:(e + 1) * 64, :],
                                in0=yps[e][0:64, :],
                                in1=rbc,
                                op=mybir.AluOpType.mult,
                            )

            def emit_c(tcw):
                # projection for t chunk tcw
                for tbl in range(4):
                    tb = 4 * tcw + tbl
                    osb = work.tile([128, C], FP32, tag="osb", bufs=2,
                                    name=f"osb{tcw}_{tbl}")
                    for ncw in range(2):
                        acc = ps.tile([128, 384], FP32, tag="a", bufs=2,
                                      name=f"c{tcw}_{tbl}_{ncw}")
                        for cc in range(NPAIR):
                            nc.tensor.matmul(
                                acc,
                                yT[cc][tcw][:, tbl * 128:(tbl + 1) * 128],
                                wp[:, cc * C + ncw * 384: cc * C + (ncw + 1) * 384],
                                start=(cc == 0),
                                stop=(cc == NPAIR - 1),
                            )
                        nc.vector.tensor_copy(osb[:, ncw * 384:(ncw + 1) * 384], acc)
                    nc.sync.dma_start(
                        out=part[tb * 128:(tb + 1) * 128, :], in_=osb
                    )

            emit_a(0)
            for tcw in range(NTC):
                emit_b(tcw)
                if tcw + 1 < NTC:
                    emit_a(tcw + 1)
                emit_c(tcw)

    nc.compile()
    return nc


def _get_nc():
    global _BUILT
    if _BUILT is None:
        _BUILT = _build_nc()
    return _BUILT


def _build_in_maps(x, W_attn, b_attn, W_proj):
    in_maps = []
    for c in range(NCORES):
        b = c // 2
        hs = (c % 2) * HPC
        q0, k0, v0 = hs * D, C + hs * D, 2 * C + hs * D
        w = HPC * D  # 384
        xT_b = np.ascontiguousarray(x[b].T).astype(ml_dtypes.bfloat16)
        wqkv_c = np.ascontiguousarray(
            np.concatenate(
                [W_attn[:, q0:q0 + w], W_attn[:, k0:k0 + w], W_attn[:, v0:v0 + w]],
                axis=1,
            )
        ).astype(ml_dtypes.bfloat16)
        bqk_c = np.stack(
            [b_attn[q0 + pp * 128: q0 + (pp + 1) * 128] for pp in range(3)]
            + [b_attn[k0 + pp * 128: k0 + (pp + 1) * 128] for pp in range(3)],
            axis=1,
        ).astype(np.float32)
        bvb_c = np.ascontiguousarray(
            np.broadcast_to(b_attn[v0:v0 + w][None, :], (128, w))
        ).astype(np.float32)
        wproj_c = np.ascontiguousarray(
            W_proj[hs * D: hs * D + w, :]
        ).astype(ml_dtypes.bfloat16)
        in_maps.append(
            {
                "xT": xT_b,
                "wqkv": wqkv_c,
                "bqk": bqk_c,
                "bvb": bvb_c,
                "wproj": wproj_c,
            }
        )
    return in_maps


def kernel(x, W_attn, b_attn, W_proj, b_proj):
    x = np.asarray(x, dtype=np.float32)
    W_attn = np.asarray(W_attn, dtype=np.float32)
    b_attn = np.asarray(b_attn, dtype=np.float32)
    W_proj = np.asarray(W_proj, dtype=np.float32)
    b_proj = np.asarray(b_proj, dtype=np.float32)

    nc = _get_nc()
    in_maps = _build_in_maps(x, W_attn, b_attn, W_proj)

    res = run_bass_kernel_spmd(nc, in_maps, core_ids=list(range(NCORES)))
    out = np.empty((B, T, C), dtype=np.float32)
    for b in range(B):
        out[b] = (
            res.results[2 * b]["part"]
            + res.results[2 * b + 1]["part"]
            + b_proj[None, :]
        )
    return out
